# revision 1
# baseline (speedup 1.0000x reference)
# Trainium2 Bass kernel for nn_BackgroundRender (equirect bilinear sample + tiny MLP).
#
# Strategy:
#   - Data-parallel over rays: B=1M rays split uniformly across 8 cores (131072 each).
#   - Host-side *sharding order*: within each core's shard, rays are grouped by
#     elevation window (29 texture rows per window, +-1 row margin) so the device
#     can fetch texels with int16-indexed dma_gather calls windowed into a
#     128MB precomputed "quad table" (quad[y*1024+x] = the 2x2 texel block
#     (y,x),(y,x+1),(y+1,x),(y+1,x+1) x 16 channels, fp32 = 256B per entry).
#   - Device computes exact fp32 angles (ACT arctan LUT + Newton rsqrt), bilinear
#     weights with zero-padding boundary handling, local int16 indices, gathers
#     quads (one 256B descriptor per ray), multiplies by the 4 bilinear weights,
#     DMA-transposes to channel-major, and runs the MLP on the PE:
#     h = relu(u @ Wstack), out = softplus(h @ W2)  with Wstack = tile(W1, 4).
#   - Host un-permutes the output back to the original ray order.
import sys

sys.path.insert(0, "/opt/trn_rl_repo")

import numpy as np
import ml_dtypes

import concourse.bass as bass
import concourse.mybir as mybir
import concourse.tile as tile
from concourse import bacc
from concourse.bass_utils import run_bass_kernel_spmd

AF = mybir.ActivationFunctionType
ALU = mybir.AluOpType
F32 = mybir.dt.float32
BF16 = mybir.dt.bfloat16
I32 = mybir.dt.int32
I16 = mybir.dt.int16

B = 1 << 20
H, W, C = 512, 1024, 16
FEATC = 128
NCORES = 8
RPC = B // NCORES

TILE = 8192
FREE = TILE // 128
WROWS = 29
NWIN = 18
QPAD_ROWS = 544

_PI = float(np.pi)
_C1 = float(W / 2) / _PI
_MAGIC = 0x5F3759DF


def _win_base_row(w: int) -> int:
    return max(29 * w - 1, 0)


# ---------------------------------------------------------------------------
# device program
# ---------------------------------------------------------------------------

def _build_program(quotas, ntiles, bench_reps=0, ablate=None):
    R = ntiles * TILE
    segs = [[] for _ in range(ntiles)]
    off = 0
    for w, q in enumerate(quotas):
        while q > 0:
            t = off // TILE
            in_tile = off - t * TILE
            take = min(TILE - in_tile, q)
            segs[t].append((in_tile // 128, (in_tile + take) // 128, w))
            off += take
            q -= take
    assert off == R

    nc = bacc.Bacc("TRN2", target_bir_lowering=False, debug=False, num_devices=NCORES, num_swdge_queues=4)

    xs = nc.dram_tensor("xs", [ntiles, 128, FREE], F32, kind="ExternalInput")
    ys = nc.dram_tensor("ys", [ntiles, 128, FREE], F32, kind="ExternalInput")
    zs = nc.dram_tensor("zs", [ntiles, 128, FREE], F32, kind="ExternalInput")
    bs = nc.dram_tensor("bs", [ntiles, 128, FREE], F32, kind="ExternalInput")
    if bench_reps:
        quad = nc.dram_tensor("quad", [QPAD_ROWS * W, 4 * C], F32)
    else:
        quad = nc.dram_tensor("quad", [QPAD_ROWS * W, 4 * C], F32, kind="ExternalInput")
    wstack = nc.dram_tensor("wstack", [64, FEATC], BF16, kind="ExternalInput")
    w2q = nc.dram_tensor("w2q", [FEATC, 128], BF16, kind="ExternalInput")
    mid = nc.dram_tensor("mid", [ntiles * 2, 12, 1024], F32)
    outd = nc.dram_tensor("out", [ntiles * 2, 12, 1024], F32, kind="ExternalOutput")

    F3 = ntiles * 2 * 12 * 1024 // 128  # final softplus free size

    with tile.TileContext(nc) as tc:
        with (
            tc.tile_pool(name="consts", bufs=1) as cpool,
            tc.tile_pool(name="inp", bufs=3) as ipool,
            tc.tile_pool(name="tmp", bufs=2) as tpool,
            tc.tile_pool(name="gat", bufs=2) as gpool,
            tc.tile_pool(name="u", bufs=2) as upool,
            tc.tile_pool(name="h", bufs=2) as hpool,
            tc.tile_pool(name="o", bufs=2) as opool,
            tc.tile_pool(name="ps", bufs=1, space="PSUM") as pspool,
            tc.tile_pool(name="pso", bufs=2, space="PSUM") as psopool,
        ):
            wst = cpool.tile([128, FEATC], BF16)
            nc.sync.dma_start(out=wst[0:64, :], in_=wstack[:])
            nc.sync.dma_start(out=wst[64:128, :], in_=wstack[:])
            w2t = cpool.tile([FEATC, 128], BF16)
            nc.sync.dma_start(out=w2t[:], in_=w2q[:])
            kone = cpool.tile([128, 1], I32)
            nc.vector.memset(kone[:], 1)
            kmagic = cpool.tile([128, 1], I32)
            nc.vector.memset(kmagic[:], _MAGIC)

            def ts(out, in0, s1, op0, s2=None, op1=None):
                if op1 is None:
                    nc.vector.tensor_scalar(out=out, in0=in0, scalar1=s1, scalar2=None, op0=op0)
                else:
                    nc.vector.tensor_scalar(out=out, in0=in0, scalar1=s1, scalar2=s2, op0=op0, op1=op1)

            def tt(out, in0, in1, op):
                nc.vector.tensor_tensor(out=out, in0=in0, in1=in1, op=op)

            gather_counter = [0]
            import contextlib
            loop_cm = tc.For_i(0, bench_reps, 1) if bench_reps else contextlib.nullcontext()
            with loop_cm:
              for t in range(ntiles):
                xt = ipool.tile([128, FREE], F32, tag="xt", name="xt")
                yt = ipool.tile([128, FREE], F32, tag="yt", name="yt")
                zt = ipool.tile([128, FREE], F32, tag="zt", name="zt")
                bt = ipool.tile([128, FREE], F32, tag="bt", name="bt")
                nc.sync.dma_start(out=xt[:], in_=xs[t])
                nc.sync.dma_start(out=yt[:], in_=ys[t])
                nc.sync.dma_start(out=zt[:], in_=zs[t])
                nc.sync.dma_start(out=bt[:], in_=bs[t])

                def tmp(tag):
                    return tpool.tile([128, FREE], F32, tag=tag, name=tag)

                # azimuth -> ix  (range-reduced arctan2: LUT input stays in [0,1])
                axp = tmp("axp")
                ts(axp[:], xt[:], -1.0, ALU.mult)
                tt(axp[:], axp[:], xt[:], ALU.max)
                ayp = tmp("ayp")
                ts(ayp[:], yt[:], -1.0, ALU.mult)
                tt(ayp[:], ayp[:], yt[:], ALU.max)
                mn = tmp("mn")
                tt(mn[:], axp[:], ayp[:], ALU.min)
                mx = tmp("mx")
                tt(mx[:], axp[:], ayp[:], ALU.max)
                ts(mx[:], mx[:], 1e-30, ALU.add)
                rmx = tmp("rmx")
                nc.vector.reciprocal_approx_fast(out=rmx[:], in_=mx[:])
                q01 = tmp("q01")
                tt(q01[:], mn[:], rmx[:], ALU.mult)
                at = tmp("at")
                nc.scalar.activation(out=at[:], in_=q01[:], func=AF.Arctan)
                # acute angle vs +x axis: a2 = swap ? pi/2 - a : a
                swp = tmp("swp")
                tt(swp[:], ayp[:], axp[:], ALU.is_gt)
                tfix = tmp("tfix")
                ts(tfix[:], at[:], -2.0, ALU.mult, float(np.pi / 2), ALU.add)
                tt(tfix[:], swp[:], tfix[:], ALU.mult)
                a2 = tmp("a2")
                tt(a2[:], at[:], tfix[:], ALU.add)
                # quadrant: phi = sy * (a2 + xneg*(pi - 2*a2))
                xneg = tmp("xneg")
                ts(xneg[:], xt[:], 0.0, ALU.is_lt)
                qf = tmp("qf")
                ts(qf[:], a2[:], -2.0, ALU.mult, _PI, ALU.add)
                tt(qf[:], xneg[:], qf[:], ALU.mult)
                tt(qf[:], a2[:], qf[:], ALU.add)
                sy = tmp("sy")
                ts(sy[:], yt[:], 0.0, ALU.is_ge)
                ts(sy[:], sy[:], 2.0, ALU.mult, -1.0, ALU.add)
                phi = tmp("phi")
                tt(phi[:], sy[:], qf[:], ALU.mult)
                ix = tmp("ix")
                ts(ix[:], phi[:], _C1, ALU.mult, float(W / 2 - 0.5), ALU.add)

                # elevation -> iy via theta = atan2(sqrt(1-z^2), z), range-reduced
                z2 = tmp("z2")
                tt(z2[:], zt[:], zt[:], ALU.mult)
                s2 = tmp("s2")
                ts(s2[:], z2[:], -1.0, ALU.mult, 1.0, ALU.add)
                ts(s2[:], s2[:], 1e-20, ALU.max)
                rs = tmp("rs")
                tt(rs.bitcast(I32)[:], s2.bitcast(I32)[:], kone[:].broadcast_to([128, FREE]), ALU.arith_shift_right)
                tt(rs.bitcast(I32)[:], kmagic[:].broadcast_to([128, FREE]), rs.bitcast(I32)[:], ALU.subtract)
                nwt = tmp("nwt")
                for _ in range(2):
                    tt(nwt[:], rs[:], rs[:], ALU.mult)
                    tt(nwt[:], nwt[:], s2[:], ALU.mult)
                    ts(nwt[:], nwt[:], -0.5, ALU.mult, 1.5, ALU.add)
                    tt(rs[:], rs[:], nwt[:], ALU.mult)
                wv = tmp("wv")
                tt(wv[:], s2[:], rs[:], ALU.mult)  # sqrt(1-z^2)
                zab = tmp("zab")
                ts(zab[:], zt[:], -1.0, ALU.mult)
                tt(zab[:], zab[:], zt[:], ALU.max)
                mn2 = tmp("mn2")
                tt(mn2[:], zab[:], wv[:], ALU.min)
                mx2 = tmp("mx2")
                tt(mx2[:], zab[:], wv[:], ALU.max)
                rmx2 = tmp("rmx2")
                nc.vector.reciprocal_approx_fast(out=rmx2[:], in_=mx2[:])
                q02 = tmp("q02")
                tt(q02[:], mn2[:], rmx2[:], ALU.mult)
                asn = tmp("asn")
                nc.scalar.activation(out=asn[:], in_=q02[:], func=AF.Arctan)
                swp2 = tmp("swp2")
                tt(swp2[:], wv[:], zab[:], ALU.is_gt)
                tfx2 = tmp("tfx2")
                ts(tfx2[:], asn[:], -2.0, ALU.mult, float(np.pi / 2), ALU.add)
                tt(tfx2[:], swp2[:], tfx2[:], ALU.mult)
                tac = tmp("tac")
                tt(tac[:], asn[:], tfx2[:], ALU.add)
                zneg = tmp("zneg")
                ts(zneg[:], zt[:], 0.0, ALU.is_lt)
                tfx3 = tmp("tfx3")
                ts(tfx3[:], tac[:], -2.0, ALU.mult, _PI, ALU.add)
                tt(tfx3[:], zneg[:], tfx3[:], ALU.mult)
                theta = tmp("theta")
                tt(theta[:], tac[:], tfx3[:], ALU.add)
                iy = tmp("iy")
                ts(iy[:], theta[:], float(H) / _PI, ALU.mult, -0.5, ALU.add)

                # floors / weights
                def floor_of(v, tagp):
                    f = tmp(tagp + "f")
                    ts(f[:], v[:], float(1 << 23), ALU.add)
                    ts(f[:], f[:], -float(1 << 23), ALU.add)
                    g_ = tmp(tagp + "g")
                    tt(g_[:], f[:], v[:], ALU.is_gt)
                    tt(f[:], f[:], g_[:], ALU.subtract)
                    return f

                x0 = floor_of(ix, "x")
                y0 = floor_of(iy, "y")
                wx1 = tmp("wx1")
                tt(wx1[:], ix[:], x0[:], ALU.subtract)
                wx0 = tmp("wx0")
                ts(wx0[:], wx1[:], -1.0, ALU.mult, 1.0, ALU.add)
                wy1 = tmp("wy1")
                tt(wy1[:], iy[:], y0[:], ALU.subtract)
                wy0 = tmp("wy0")
                ts(wy0[:], wy1[:], -1.0, ALU.mult, 1.0, ALU.add)

                def bounds(v0, wlo, whi, hi, tagp):
                    mlo = tmp(tagp + "mlo")
                    ts(mlo[:], v0[:], 0.0, ALU.is_lt)
                    mhi = tmp(tagp + "mhi")
                    ts(mhi[:], v0[:], float(hi + 1), ALU.is_ge)
                    sm = tmp(tagp + "sm")
                    tt(sm[:], mlo[:], mhi[:], ALU.add)
                    ts(sm[:], sm[:], -1.0, ALU.mult, 1.0, ALU.add)
                    a0 = tmp(tagp + "a0")
                    tt(a0[:], wlo[:], sm[:], ALU.mult)
                    tl = tmp(tagp + "tl")
                    tt(tl[:], mlo[:], whi[:], ALU.mult)
                    tt(a0[:], a0[:], tl[:], ALU.add)
                    a1 = tmp(tagp + "a1")
                    tt(a1[:], whi[:], sm[:], ALU.mult)
                    th_ = tmp(tagp + "th")
                    tt(th_[:], mhi[:], wlo[:], ALU.mult)
                    tt(a1[:], a1[:], th_[:], ALU.add)
                    vq = tmp(tagp + "vq")
                    ts(vq[:], v0[:], 0.0, ALU.max, float(hi), ALU.min)
                    return a0, a1, vq

                ax0, ax1, xq = bounds(x0, wx0, wx1, W - 2, "bx")
                by0, by1, yq = bounds(y0, wy0, wy1, H - 2, "by")

                w4 = tpool.tile([128, FREE, 4], F32, tag="w4", name="w4")
                tt(w4[:, :, 0], by0[:], ax0[:], ALU.mult)
                tt(w4[:, :, 1], by0[:], ax1[:], ALU.mult)
                tt(w4[:, :, 2], by1[:], ax0[:], ALU.mult)
                tt(w4[:, :, 3], by1[:], ax1[:], ALU.mult)

                # local int16 gather index
                flat = tmp("flat")
                ts(flat[:], yq[:], float(W), ALU.mult)
                tt(flat[:], flat[:], xq[:], ALU.add)
                tt(flat[:], flat[:], bt[:], ALU.subtract)
                ts(flat[:], flat[:], 0.0, ALU.max, 32767.0, ALU.min)
                idx16 = tpool.tile([128, FREE], I16, tag="idx16", name="idx16")
                nc.vector.tensor_copy(out=idx16[:], in_=flat[:])

                idxw = tpool.tile([128, FREE, 8], I16, tag="idxw", name="idxw")
                nc.vector.memset(idxw[:], 0)
                for j in range(8):
                    nc.sync.dma_start(out=idxw[0:16, :, j], in_=idx16[16 * j : 16 * j + 16, :])
                nc.sync.dma_start(out=idxw[16:32, :, :], in_=idxw[0:16, :, :])
                nc.sync.dma_start(out=idxw[32:64, :, :], in_=idxw[0:32, :, :])
                nc.sync.dma_start(out=idxw[64:128, :, :], in_=idxw[0:64, :, :])

                # gather
                g = gpool.tile([128, FREE, 4 * C], F32, tag="g", name="g")
                for segi, (flo, fhi, w) in enumerate(segs[t] if ablate != "nogather" else []):
                    gather_counter[0] += 1
                    ni = (fhi - flo) * 128
                    base = _win_base_row(w) * W
                    nc.gpsimd.dma_gather(
                        out_ap=g[:, flo:fhi, :],
                        in_ap=quad[base : base + 32768, :],
                        idxs_ap=idxw[:, flo:fhi, :].rearrange("p a b -> p (a b)"),
                        num_idxs=ni,
                        num_idxs_reg=ni,
                        elem_size=4 * C,
                        single_packet=False,
                        queue_num=(gather_counter[0] - 1) % 4,
                    )

                if ablate == "gatheronly":
                    nc.sync.dma_start(out=mid[2 * t, 0:1, 0:64], in_=g[0:1, 0:1, :])
                    continue
                # weighted quad -> u (bf16)
                u = upool.tile([128, FREE, 4, C], BF16, tag="u", name="u")
                g4 = g[:].rearrange("p f (j c) -> p f j c", j=4)
                tt(u[:], g4, w4[:, :, :, None].broadcast_to([128, FREE, 4, C]), ALU.mult)

                # transpose to channel-major
                uflat = u[:].rearrange("p f j c -> p (f j c)")
                uT = upool.tile([128, 32, 128], BF16, tag="uT", name="uT")
                for k in range(32):
                    nc.sync.dma_start(
                        out=uT[:, k, :],
                        in_=uflat[:, 128 * k : 128 * (k + 1)],
                        transpose=True,
                    )

                # MLP
                for half in range(2):
                    pso = psopool.tile([128, 1024], F32, tag="pso", name="pso")
                    for quarter in range(2):
                        qq = half * 2 + quarter
                        psA = pspool.tile([128, 1024], F32, tag="psA", name="psA")
                        psB = pspool.tile([128, 1024], F32, tag="psB", name="psB")
                        for j in range(8):
                            k = qq * 8 + j
                            nc.tensor.matmul(
                                out=psA[:, 128 * j : 128 * (j + 1)],
                                lhsT=wst[0:64, :],
                                rhs=uT[0:64, k, :],
                                start=True, stop=True,
                                tile_position=(0, 0),
                            )
                            nc.tensor.matmul(
                                out=psB[:, 128 * j : 128 * (j + 1)],
                                lhsT=wst[64:128, :],
                                rhs=uT[64:128, k, :],
                                start=True, stop=True,
                                tile_position=(64, 0),
                            )
                        hq = hpool.tile([128, 2048], BF16, tag="hq", name="hq")
                        nc.scalar.activation(out=hq[:, 0:1024], in_=psA[:], func=AF.Relu)
                        nc.vector.tensor_scalar(
                            out=hq[:, 1024:2048], in0=psB[:],
                            scalar1=0.0, scalar2=None, op0=ALU.max,
                        )
                        for side in range(2):
                            c = 2 * quarter + side
                            for ns in range(2):
                                nc.tensor.matmul(
                                    out=pso[32 * c : 32 * c + 32, 512 * ns : 512 * (ns + 1)],
                                    lhsT=w2t[:, 32 * c : 32 * c + 32],
                                    rhs=hq[:, 1024 * side + 512 * ns : 1024 * side + 512 * (ns + 1)],
                                    start=True, stop=True,
                                    tile_position=(0, 32 * c),
                                )
                    so = opool.tile([128, 1024], F32, tag="so", name="so")
                    nc.scalar.activation(out=so[:], in_=pso[:], func=AF.Copy)
                    for c in range(4):
                        nc.sync.dma_start(
                            out=mid[2 * t + half, 3 * c : 3 * c + 3, :],
                            in_=so[32 * c : 32 * c + 3, :],
                        )

            # final softplus pass (exp/ln table loaded once)
            fin = opool.tile([128, F3], F32, tag="fin", name="fin")
            nc.sync.dma_start(out=fin[:], in_=mid[:].rearrange("a b c -> (a b c)").rearrange("(p f) -> p f", p=128))
            nc.scalar.activation(out=fin[:], in_=fin[:], func=AF.Exp)
            nc.scalar.activation(out=fin[:], in_=fin[:], func=AF.Ln, bias=1.0)
            nc.sync.dma_start(out=outd[:].rearrange("a b c -> (a b c)").rearrange("(p f) -> p f", p=128), in_=fin[:])

    nc.compile()
    return nc, segs


# ---------------------------------------------------------------------------
# host side
# ---------------------------------------------------------------------------

_CACHE = {}


def _quad_table(bg_mat):
    t = np.ascontiguousarray(np.transpose(bg_mat, (1, 2, 0)))  # [H, W, C]
    q = np.zeros((QPAD_ROWS, W, 4, C), np.float32)
    q[:H, :, 0, :] = t
    q[:H, : W - 1, 1, :] = t[:, 1:]
    q[: H - 1, :, 2, :] = t[1:]
    q[: H - 1, : W - 1, 3, :] = t[1:, 1:]
    return q.reshape(QPAD_ROWS * W, 4 * C)


def _prepare(viewdirs, bg_mat, W1, W2):
    vd = np.asarray(viewdirs, np.float32)
    nrays = vd.shape[0]
    rpc = nrays // NCORES

    z = np.clip(vd[:, 2].astype(np.float64), -1.0, 1.0)
    iy = np.arccos(z) * (H / np.pi) - 0.5
    yq = np.clip(np.floor(iy), 0, H - 2).astype(np.int64)
    win = np.minimum(yq // WROWS, NWIN - 1)

    counts = np.zeros((NCORES, NWIN), np.int64)
    orders = []
    for c in range(NCORES):
        wslice = win[c * rpc : (c + 1) * rpc]
        order = np.argsort(wslice, kind="stable")
        orders.append(order)
        counts[c] = np.bincount(wslice, minlength=NWIN)

    quotas = [int(-(-counts[:, w].max() // 128) * 128) for w in range(NWIN)]
    R = sum(quotas)
    ntiles = -(-R // TILE)
    quotas[-1] += ntiles * TILE - R
    R = ntiles * TILE

    woff = np.zeros(NWIN + 1, np.int64)
    np.cumsum(quotas, out=woff[1:])
    quad = _quad_table(np.asarray(bg_mat, np.float32))
    wstack = np.tile(np.asarray(W1, np.float32), (4, 1)).astype(ml_dtypes.bfloat16)
    w2q = np.zeros((FEATC, 128), np.float32)
    for c in range(4):
        w2q[:, 32 * c : 32 * c + 3] = np.asarray(W2, np.float32)
    w2q = w2q.astype(ml_dtypes.bfloat16)
    base_row = np.array([_win_base_row(w) * W for w in range(NWIN)], np.float32)

    in_maps, slotmaps = [], []
    for c in range(NCORES):
        order = orders[c]
        cw = counts[c]
        slots = np.full(R, -1, np.int64)
        basef = np.zeros(R, np.float32)
        pos = 0
        for w in range(NWIN):
            n = int(cw[w])
            slots[woff[w] : woff[w] + n] = c * rpc + order[pos : pos + n]
            basef[woff[w] : woff[w + 1]] = base_row[w]
            pos += n
        slotmaps.append(slots)

        dirs = np.empty((R, 3), np.float32)
        dirs[:] = (1.0, 0.0, 0.0)
        valid = slots >= 0
        dirs[valid] = vd[slots[valid]]

        def swz(a):
            return np.ascontiguousarray(a.reshape(ntiles, FREE, 128).transpose(0, 2, 1))

        in_maps.append(
            {
                "xs": swz(dirs[:, 0].copy()),
                "ys": swz(dirs[:, 1].copy()),
                "zs": swz(dirs[:, 2].copy()),
                "bs": swz(basef),
                "quad": quad,
                "wstack": wstack,
                "w2q": w2q,
            }
        )
    return (tuple(quotas), ntiles), in_maps, slotmaps, nrays


def _decode(results, slotmaps, ntiles, nrays):
    th = np.arange(ntiles * 2)[:, None, None]
    cc = (np.arange(12)[None, :, None]) // 3
    ch = (np.arange(12)[None, :, None]) % 3
    col = np.arange(1024)[None, None, :]
    qq = 2 * (th % 2) + cc // 2
    side = cc % 2
    ns = col // 512
    rem = col % 512
    k = 8 * qq + 4 * ns + rem // 128
    slot = (th // 2) * TILE + 256 * k + 128 * side + (rem % 128)

    out = np.zeros((nrays, 3), np.float32)
    chf = np.broadcast_to(ch, slot.shape).reshape(-1)
    flat_slot = slot.reshape(-1)
    for c in range(len(slotmaps)):
        o = np.asarray(results[c]["out"])
        orig = slotmaps[c][flat_slot]
        m = orig >= 0
        out[orig[m], chf[m]] = o.reshape(-1)[m]
    return out


def kernel(viewdirs, roughness, bg_mat, W1, W2):
    del roughness  # unused by the reference model
    key_parts, in_maps, slotmaps, nrays = _prepare(viewdirs, bg_mat, W1, W2)
    quotas, ntiles = key_parts
    if key_parts not in _CACHE:
        _CACHE[key_parts] = _build_program(list(quotas), ntiles)
    nc, _segs = _CACHE[key_parts]
    res = run_bass_kernel_spmd(nc, in_maps, list(range(NCORES)))
    return _decode(res.results, slotmaps, ntiles, nrays)

